# revision 16
# baseline (speedup 1.0000x reference)
"""Multi-Head Latent Attention (MLA) Bass/Tile kernel for 8 TRN2 NeuronCores.

Sharding: 2-way data-parallel over batch x 4-way tensor-parallel over heads.
Core c = (b, g) with b = c // 4, g = c % 4 owns batch b and heads 4g..4g+3.
Each core computes a partial (S, D) output (its heads' contribution through
wo); the host sums the 4 head-group partials per batch.

Device-side plan per core (all matmuls f32r unless noted):
  phase 1: latentT = w_kv_down.T @ x.T   [DL, S]   (spilled to DRAM)
           q       = x @ wq_g            [S, HG*DK] -> rmsnorm -> rope
           qT      = per-head PE transpose -> [DK, S] per head
  phase 2: k_rope/k_nope/v from latentT; rope+concat+rmsnorm k; kT transpose
  phase 3: per (head, 512-col sq chunk): scoresT = kT.T @ qT  [sk, sq] in
           PSUM; exp on ACT (scale 1/sqrt(DK), no max subtraction -- scores
           are bounded ~±6 so fp32 exp is safe); causal handled by skipping
           sk tiles > sq and one affine_select on the diagonal strip;
           oT += v.T @ probsT; rowsum via ones-vector matmul; normalize oT
           with broadcast reciprocal; out_partial = sum_h oT_h.T @ wo_h.
"""

import os
import sys
from contextlib import ExitStack

import numpy as np

for _p in ("/opt/trn_rl_repo", os.path.expanduser("~/.axon_site/_ro/trn_rl_repo")):
    if os.path.isdir(_p) and _p not in sys.path:
        sys.path.append(_p)

import concourse.bass as bass
import concourse.bacc as bacc
import concourse.mybir as mybir
import concourse.tile as tile
from concourse.masks import make_identity

F32 = mybir.dt.float32
F32R = mybir.dt.float32r
AX = mybir.AxisListType
ALU = mybir.AluOpType
ACTF = mybir.ActivationFunctionType

# Problem constants (hardcoded for nn_MultiHeadLatentAttention_74904229642374)
B, S, D, H, DK, DL, DR = 2, 2048, 2048, 16, 128, 512, 64
EPS = 1e-6
NCORES = 8
GROUPS = 4            # head-group (tensor-parallel) dimension
HG = H // GROUPS      # heads per core
HD = HG * DK          # per-core head width of q/v/wo
PT = 128              # partition tile
SCH = 512             # sequence chunk width

USE_F32R = True
MMDT = F32R if USE_F32R else F32


def _r(ap):
    return ap


def build_nc(s=S, d=D, dl=DL, dbg=False):
    """Build the per-core Bass program. s/d/dl shrinkable for sim tests."""
    nsq = s // PT          # number of 128-row seq tiles
    nch = s // SCH         # number of 512-wide seq chunks
    nkd = d // PT          # D contraction chunks
    ndl = dl // PT         # DL contraction chunks
    spc = SCH // PT        # seq tiles per chunk (4)
    now = min(SCH, d)      # wo output chunk width
    nno = d // now         # output col chunks of wo matmul

    nc = bacc.Bacc("TRN2", target_bir_lowering=False, debug=False,
                   num_devices=NCORES)

    xt_d = nc.dram_tensor("xt", [d, s], MMDT, kind="ExternalInput")
    wq_d = nc.dram_tensor("wq", [d, HD], MMDT, kind="ExternalInput")
    wkv_d = nc.dram_tensor("wkv", [d, dl], MMDT, kind="ExternalInput")
    wkr_d = nc.dram_tensor("wkr", [dl, HG * DR], MMDT, kind="ExternalInput")
    wkn_d = nc.dram_tensor("wkn", [dl, HG * (DK - DR)], MMDT, kind="ExternalInput")
    wv_d = nc.dram_tensor("wv", [dl, HD], MMDT, kind="ExternalInput")
    wo_d = nc.dram_tensor("wo", [HD, d], MMDT, kind="ExternalInput")
    cos_d = nc.dram_tensor("cos", [s, DR // 2], F32, kind="ExternalInput")
    sin_d = nc.dram_tensor("sin", [s, DR // 2], F32, kind="ExternalInput")
    qnw_d = nc.dram_tensor("qnw", [DK, 1], F32, kind="ExternalInput")
    knw_d = nc.dram_tensor("knw", [DK, 1], F32, kind="ExternalInput")
    out_d = nc.dram_tensor("out", [s, d], F32, kind="ExternalOutput")
    lat_d = nc.dram_tensor("lat_spill", [dl, s], MMDT)  # internal DRAM
    if dbg:
        dbg_qt = nc.dram_tensor("dbg_qt", [PT, HG * s], MMDT, kind="ExternalOutput")
        dbg_kt = nc.dram_tensor("dbg_kt", [PT, HG * s], MMDT, kind="ExternalOutput")
        dbg_v = nc.dram_tensor("dbg_v", [PT, (s // PT) * HD], MMDT, kind="ExternalOutput")
        dbg_ot = nc.dram_tensor("dbg_ot", [PT, (s // SCH) * HG * SCH], MMDT, kind="ExternalOutput")
        dbg_rs = nc.dram_tensor("dbg_rs", [(s // SCH) * HG, SCH], F32, kind="ExternalOutput")

    scale = 1.0 / float(np.sqrt(np.float32(DK)))

    with tile.TileContext(nc) as tc, ExitStack() as ctx:
        const = ctx.enter_context(tc.tile_pool(name="const", bufs=1))
        qt_pool = ctx.enter_context(tc.tile_pool(name="qt", bufs=1))
        stat = ctx.enter_context(tc.tile_pool(name="stat", bufs=4))

        ident = const.tile([PT, PT], F32)
        make_identity(nc, ident[:])
        ones_f = const.tile([PT, 1], F32)
        nc.gpsimd.memset(ones_f[:], 1.0)
        ones = const.tile([PT, 1], MMDT)
        nc.vector.tensor_copy(ones[:], ones_f[:])
        qnw = const.tile([PT, 1], F32)
        nc.sync.dma_start(out=qnw[:], in_=qnw_d.ap())
        knw = const.tile([PT, 1], F32)
        nc.sync.dma_start(out=knw[:], in_=knw_d.ap())
        cos_sb = const.tile([PT, nsq, DR // 2], F32)
        sin_sb = const.tile([PT, nsq, DR // 2], F32)
        cos_r = cos_d.ap().rearrange("(t p) f -> t p f", p=PT)
        sin_r = sin_d.ap().rearrange("(t p) f -> t p f", p=PT)
        for t in range(nsq):
            nc.sync.dma_start(out=cos_sb[:, t, :], in_=cos_r[t])
            nc.sync.dma_start(out=sin_sb[:, t, :], in_=sin_r[t])

        qT = qt_pool.tile([PT, HG, s], MMDT)   # [dk, h, sq]

        def rmsnorm_rinv(psum_aps, width):
            """psum_aps: [128, HG, w] APs; returns rinv [128, HG] tile
            holding 1/sqrt(mean-over-DK of squares + eps). Uses ACT Square
            with accum_out (single PSUM read per op)."""
            parts = []
            for ap in psum_aps:
                pt_ = stat.tile([PT, HG], F32, tag="sspart")
                sq = stat.tile([PT, ap.shape[2]], F32, tag="sq")
                for h in range(HG):
                    nc.scalar.activation(sq[:], ap[:, h, :], ACTF.Square,
                                         accum_out=pt_[:, h:h + 1])
                parts.append(pt_)
            if len(parts) == 1:
                ss = parts[0]
            else:
                ss = stat.tile([PT, HG], F32, tag="ss")
                nc.vector.tensor_add(ss[:], parts[0][:], parts[1][:])
            nc.vector.tensor_scalar(ss[:], ss[:], 1.0 / DK, EPS,
                                    op0=ALU.mult, op1=ALU.add)
            nc.scalar.sqrt(ss[:], ss[:])
            rinv = stat.tile([PT, HG], F32, tag="rinv")
            nc.vector.reciprocal(rinv[:], ss[:])
            return rinv

        def rope_block(dst, x1, x2, st, tmp_pool):
            """dst [128, HG, DR] target; x1/x2 [128, HG, DR//2] first/second
            half source APs; st = seq tile index."""
            half = DR // 2
            c = cos_sb[:, st, :].unsqueeze(1).broadcast_to([PT, HG, half])
            sn = sin_sb[:, st, :].unsqueeze(1).broadcast_to([PT, HG, half])
            t1 = tmp_pool.tile([PT, HG, half], F32, tag="rt1")
            t2 = tmp_pool.tile([PT, HG, half], F32, tag="rt2")
            t3 = tmp_pool.tile([PT, HG, half], F32, tag="rt3")
            t4 = tmp_pool.tile([PT, HG, half], F32, tag="rt4")
            nc.vector.tensor_mul(t1[:], x1, c)
            nc.vector.tensor_mul(t2[:], x2, sn)
            nc.vector.tensor_mul(t3[:], x1, sn)
            nc.vector.tensor_mul(t4[:], x2, c)
            nc.vector.tensor_add(dst[:, :, 0:half], t1[:], t2[:])
            nc.vector.tensor_sub(dst[:, :, half:DR], t4[:], t3[:])

        # ---------------- phase 1: latentT (to DRAM) + qT ----------------
        with tc.tile_pool(name="wq", bufs=1) as wq_pool, \
             tc.tile_pool(name="wkv", bufs=1) as wkv_pool, \
             tc.tile_pool(name="xt", bufs=nkd + 2) as xt_pool, \
             tc.tile_pool(name="latst", bufs=3) as latst_pool, \
             tc.tile_pool(name="ps1", bufs=3, space="PSUM") as ps1, \
             tc.tile_pool(name="pst", bufs=2, space="PSUM") as pst, \
             tc.tile_pool(name="qn", bufs=2) as qn_pool, \
             tc.tile_pool(name="rtmp", bufs=2) as rtmp:

            wq_sb = wq_pool.tile([PT, nkd, HD], MMDT)
            wq_r = wq_d.ap().rearrange("(k p) n -> k p n", p=PT)
            wkv_sb = wkv_pool.tile([PT, nkd, dl], MMDT)
            wkv_r = wkv_d.ap().rearrange("(k p) n -> k p n", p=PT)
            for k in range(nkd):
                nc.sync.dma_start(out=wq_sb[:, k, :], in_=wq_r[k])
                nc.sync.dma_start(out=wkv_sb[:, k, :], in_=wkv_r[k])

            xt_r = xt_d.ap().rearrange("k (c ss) -> c k ss", ss=SCH) \
                .rearrange("c (k p) ss -> c k p ss", p=PT)
            for c in range(nch):
                xts = []
                for k in range(nkd):
                    xk = xt_pool.tile([PT, SCH], MMDT, tag="xt")
                    nc.sync.dma_start(out=xk[:], in_=xt_r[c, k])
                    xts.append(xk)
                for dt_ in range(ndl):
                    pl = ps1.tile([PT, SCH], F32, tag="pslat")
                    for k in range(nkd):
                        nc.tensor.matmul(
                            pl[:],
                            _r(wkv_sb[:, k, dt_ * PT:(dt_ + 1) * PT]),
                            _r(xts[k][:]),
                            start=(k == 0), stop=(k == nkd - 1))
                    lst = latst_pool.tile([PT, SCH], MMDT, tag="latst")
                    nc.scalar.copy(lst[:], pl[:])
                    nc.sync.dma_start(
                        out=lat_d.ap()[dt_ * PT:(dt_ + 1) * PT,
                                       c * SCH:(c + 1) * SCH],
                        in_=lst[:])
                for t in range(spc):
                    st = c * spc + t
                    pq = ps1.tile([PT, HD], F32, tag="psq")
                    for k in range(nkd):
                        nc.tensor.matmul(
                            pq[:],
                            _r(xts[k][:, t * PT:(t + 1) * PT]),
                            _r(wq_sb[:, k, :]),
                            start=(k == 0), stop=(k == nkd - 1))
                    pqv = pq[:].rearrange("p (h e) -> p h e", h=HG)
                    rinv = rmsnorm_rinv([pqv], DK)
                    qn = qn_pool.tile([PT, HG, DK], F32, tag="qn")
                    for h in range(HG):
                        nc.vector.tensor_scalar(
                            qn[:, h, :], pq[:, h * DK:(h + 1) * DK],
                            rinv[:, h:h + 1], None, op0=ALU.mult)
                    rope_block(qn[:, :, 0:DR], qn[:, :, 0:DR // 2],
                               qn[:, :, DR // 2:DR], st, rtmp)
                    for h in range(HG):
                        tp = pst.tile([PT, PT], F32, tag="tp")
                        nc.tensor.transpose(tp[:], qn[:, h, :], ident[:])
                        nc.vector.tensor_scalar(
                            qT[:, h, st * PT:(st + 1) * PT], tp[:],
                            qnw[:, 0:1], None, op0=ALU.mult)

        kt_pool = ctx.enter_context(tc.tile_pool(name="kt", bufs=1))
        v_pool = ctx.enter_context(tc.tile_pool(name="v", bufs=1))
        kT = kt_pool.tile([PT, HG, s], MMDT)         # [dk, h, sk]
        v_sb = v_pool.tile([PT, nsq, HG, DK], MMDT)  # [sk-in-tile, tile, h, dk]

        # ---------------- phase 2: k and v from latentT ----------------
        with tc.tile_pool(name="wkr", bufs=1) as wkr_pool, \
             tc.tile_pool(name="wkn", bufs=1) as wkn_pool, \
             tc.tile_pool(name="wv", bufs=1) as wv_pool, \
             tc.tile_pool(name="lat2", bufs=ndl + 2) as lat2_pool, \
             tc.tile_pool(name="ps2", bufs=2, space="PSUM") as ps2, \
             tc.tile_pool(name="pst2", bufs=2, space="PSUM") as pst2, \
             tc.tile_pool(name="kn", bufs=2) as kn_pool, \
             tc.tile_pool(name="rtmp2", bufs=2) as rtmp2:

            wkr_sb = wkr_pool.tile([PT, ndl, HG * DR], MMDT)
            wkr_r = wkr_d.ap().rearrange("(k p) n -> k p n", p=PT)
            wkn_sb = wkn_pool.tile([PT, ndl, HG * (DK - DR)], MMDT)
            wkn_r = wkn_d.ap().rearrange("(k p) n -> k p n", p=PT)
            wv_sb = wv_pool.tile([PT, ndl, HD], MMDT)
            wv_r = wv_d.ap().rearrange("(k p) n -> k p n", p=PT)
            for k in range(ndl):
                nc.sync.dma_start(out=wkr_sb[:, k, :], in_=wkr_r[k])
                nc.sync.dma_start(out=wkn_sb[:, k, :], in_=wkn_r[k])
                nc.sync.dma_start(out=wv_sb[:, k, :], in_=wv_r[k])

            lat_r = lat_d.ap().rearrange("k (c ss) -> c k ss", ss=SCH) \
                .rearrange("c (k p) ss -> c k p ss", p=PT)
            for c in range(nch):
                lts = []
                for k in range(ndl):
                    lk = lat2_pool.tile([PT, SCH], MMDT, tag="lat2")
                    nc.sync.dma_start(out=lk[:], in_=lat_r[c, k])
                    lts.append(lk)
                for t in range(spc):
                    st = c * spc + t
                    pkr = ps2.tile([PT, HG * DR], F32, tag="pskr")
                    pkn = ps2.tile([PT, HG * (DK - DR)], F32, tag="pskn")
                    pv = ps2.tile([PT, HD], F32, tag="psv")
                    for k in range(ndl):
                        lt = _r(lts[k][:, t * PT:(t + 1) * PT])
                        nc.tensor.matmul(pkr[:], lt, _r(wkr_sb[:, k, :]),
                                         start=(k == 0), stop=(k == ndl - 1))
                        nc.tensor.matmul(pkn[:], lt, _r(wkn_sb[:, k, :]),
                                         start=(k == 0), stop=(k == ndl - 1))
                        nc.tensor.matmul(pv[:], lt, _r(wv_sb[:, k, :]),
                                         start=(k == 0), stop=(k == ndl - 1))
                    pkrv = pkr[:].rearrange("p (h e) -> p h e", h=HG)
                    pknv = pkn[:].rearrange("p (h e) -> p h e", h=HG)
                    rinv = rmsnorm_rinv([pkrv, pknv], DK)
                    kn = kn_pool.tile([PT, HG, DK], F32, tag="kn")
                    rope_block(kn[:, :, 0:DR], pkrv[:, :, 0:DR // 2],
                               pkrv[:, :, DR // 2:DR], st, rtmp2)
                    nc.vector.tensor_copy(kn[:, :, DR:DK], pknv)
                    for h in range(HG):
                        nc.vector.tensor_scalar(
                            kn[:, h, :], kn[:, h, :],
                            rinv[:, h:h + 1], None, op0=ALU.mult)
                        tp = pst2.tile([PT, PT], F32, tag="tp2")
                        nc.tensor.transpose(tp[:], kn[:, h, :], ident[:])
                        nc.vector.tensor_scalar(
                            kT[:, h, st * PT:(st + 1) * PT], tp[:],
                            knw[:, 0:1], None, op0=ALU.mult)
                    nc.scalar.copy(
                        v_sb[:, st, :, :].rearrange("p h e -> p (h e)"), pv[:])

        if dbg:
            nc.sync.dma_start(out=dbg_qt.ap(),
                              in_=qT[:].rearrange("p h s -> p (h s)"))
            nc.sync.dma_start(out=dbg_kt.ap(),
                              in_=kT[:].rearrange("p h s -> p (h s)"))
            nc.sync.dma_start(out=dbg_v.ap(),
                              in_=v_sb[:].rearrange("p t h e -> p (t h e)"))

        # ---------------- phase 3: attention + wo ----------------
        with tc.tile_pool(name="wo", bufs=1) as wo_pool, \
             tc.tile_pool(name="probs", bufs=3) as probs_pool, \
             tc.tile_pool(name="ot", bufs=2) as ot_pool, \
             tc.tile_pool(name="bcast", bufs=2) as bc_pool, \
             tc.tile_pool(name="outst", bufs=3) as out_pool, \
             tc.tile_pool(name="pssc", bufs=2, space="PSUM") as pssc, \
             tc.tile_pool(name="psot", bufs=2, space="PSUM") as psot, \
             tc.tile_pool(name="psrs", bufs=2, space="PSUM") as psrs, \
             tc.tile_pool(name="pswo", bufs=2, space="PSUM") as pswo:

            wo_sb = wo_pool.tile([PT, HG, d], MMDT)
            wo_r = wo_d.ap().rearrange("(h p) n -> h p n", p=PT)
            for h in range(HG):
                nc.sync.dma_start(out=wo_sb[:, h, :], in_=wo_r[h])

            for cj in range(nch):
                ot_sb = ot_pool.tile([PT, HG, SCH], MMDT, tag="otsb")
                for h in range(HG):
                    po = psot.tile([PT, SCH], F32, tag="psot")
                    prs = psrs.tile([1, SCH], F32, tag="psrs")
                    nsk = spc * cj + spc
                    for i in range(nsk):
                        dg = i - spc * cj
                        c0 = 0 if dg < 0 else min(PT * dg, SCH - 256)
                        w = SCH - c0
                        psc = pssc.tile([PT, SCH], F32, tag="pssc")
                        nc.tensor.matmul(
                            psc[:, c0:SCH],
                            _r(kT[:, h, i * PT:(i + 1) * PT]),
                            _r(qT[:, h, cj * SCH + c0:(cj + 1) * SCH]),
                            start=True, stop=True)
                        pb = probs_pool.tile([PT, SCH], MMDT, tag="probs")
                        nc.scalar.activation(pb[:, c0:SCH], psc[:, c0:SCH],
                                             ACTF.Exp, scale=scale)
                        if dg >= 0:
                            nc.gpsimd.affine_select(
                                out=pb[:, c0:SCH], in_=pb[:, c0:SCH],
                                compare_op=ALU.is_ge, fill=0.0,
                                base=SCH * cj + c0 - PT * i,
                                pattern=[[1, w]], channel_multiplier=-1)
                        nc.tensor.matmul(
                            po[:, c0:SCH],
                            _r(v_sb[:, i, h, :]),
                            _r(pb[:, c0:SCH]),
                            start=(i == 0), stop=(i == nsk - 1))
                        nc.tensor.matmul(
                            prs[:, c0:SCH],
                            _r(ones[:]),
                            _r(pb[:, c0:SCH]),
                            start=(i == 0), stop=(i == nsk - 1))
                    # normalize oT by broadcast reciprocal of rowsums
                    rs_sb = bc_pool.tile([1, SCH], F32, tag="rssb")
                    nc.scalar.copy(rs_sb[:], prs[:])
                    bc = bc_pool.tile([PT, SCH], F32, tag="bcast")
                    nc.gpsimd.partition_broadcast(bc[:], rs_sb[:], channels=PT)
                    rec = bc_pool.tile([PT, SCH], F32, tag="rec")
                    nc.vector.reciprocal(rec[:], bc[:])
                    nc.vector.tensor_mul(ot_sb[:, h, :], po[:], rec[:])
                    if dbg:
                        nc.sync.dma_start(
                            out=dbg_rs.ap()[cj * HG + h:cj * HG + h + 1, :],
                            in_=rs_sb[:])
                if dbg:
                    nc.sync.dma_start(
                        out=dbg_ot.ap()[:, cj * HG * SCH:(cj + 1) * HG * SCH],
                        in_=ot_sb[:].rearrange("p h s -> p (h s)"))
                # wo for this chunk
                for t in range(spc):
                    st = cj * spc + t
                    for n in range(nno):
                        pw = pswo.tile([PT, now], F32, tag="pswo")
                        for h in range(HG):
                            nc.tensor.matmul(
                                pw[:],
                                _r(ot_sb[:, h, t * PT:(t + 1) * PT]),
                                _r(wo_sb[:, h, n * now:(n + 1) * now]),
                                start=(h == 0), stop=(h == HG - 1))
                        ob = out_pool.tile([PT, now], F32, tag="outst")
                        nc.scalar.copy(ob[:], pw[:])
                        nc.sync.dma_start(
                            out=out_d.ap()[st * PT:(st + 1) * PT,
                                           n * now:(n + 1) * now],
                            in_=ob[:])

    nc.compile()
    return nc


def rope_tables(s):
    quarter = DR // 4
    freq = (1.0 / 10000.0) ** np.linspace(0.0, 1.0, quarter, dtype=np.float32)
    freq = np.concatenate([freq, np.zeros((quarter,), np.float32)])
    theta = np.arange(s, dtype=np.float32)[:, None] * freq[None, :]
    return np.cos(theta).astype(np.float32), np.sin(theta).astype(np.float32)


def make_in_maps(x, wq, w_kv_down, w_k_rope, w_k_nope, wv, wo,
                 q_norm_w, k_norm_w):
    s = x.shape[1]
    cos, sin = rope_tables(s)
    ca = np.ascontiguousarray
    in_maps = []
    for c in range(NCORES):
        b, g = divmod(c, GROUPS)
        in_maps.append({
            "xt": ca(x[b].T).astype(np.float32),
            "wq": ca(wq[:, g * HD:(g + 1) * HD]),
            "wkv": ca(w_kv_down),
            "wkr": ca(w_k_rope[:, g * HG * DR:(g + 1) * HG * DR]),
            "wkn": ca(w_k_nope[:, g * HG * (DK - DR):(g + 1) * HG * (DK - DR)]),
            "wv": ca(wv[:, g * HD:(g + 1) * HD]),
            "wo": ca(wo[g * HD:(g + 1) * HD, :]),
            "cos": cos, "sin": sin,
            "qnw": ca(q_norm_w.reshape(DK, 1)),
            "knw": ca(k_norm_w.reshape(DK, 1)),
        })
    return in_maps


_NC_CACHE = {}


def run(inputs, trace=False, **kwargs):
    from concourse.bass_utils import run_bass_kernel_spmd
    if "nc" not in _NC_CACHE:
        _NC_CACHE["nc"] = build_nc()
    nc = _NC_CACHE["nc"]
    in_maps = make_in_maps(**inputs)
    res = run_bass_kernel_spmd(nc, in_maps, core_ids=list(range(NCORES)),
                               trace=trace, **kwargs)
    outs = [r["out"] for r in res.results]
    full = np.empty((B, S, D), np.float32)
    for b in range(B):
        full[b] = outs[b * GROUPS]
        for g in range(1, GROUPS):
            full[b] += outs[b * GROUPS + g]
    return full, res


def kernel(**inputs):
    out, _ = run(inputs)
    return out


# revision 17
# speedup vs baseline: 33.0318x; 33.0318x over previous
"""Multi-Head Latent Attention (MLA) Bass/Tile kernel for 8 TRN2 NeuronCores.

Sharding: 2-way data-parallel over batch x 4-way tensor-parallel over heads.
Core c = (b, g) with b = c // 4, g = c % 4 owns batch b and heads 4g..4g+3.
Each core computes a partial (S, D) output (its heads' contribution through
wo); the host sums the 4 head-group partials per batch.

Device-side plan per core (all matmuls f32r unless noted):
  phase 1: latentT = w_kv_down.T @ x.T   [DL, S]   (spilled to DRAM)
           q       = x @ wq_g            [S, HG*DK] -> rmsnorm -> rope
           qT      = per-head PE transpose -> [DK, S] per head
  phase 2: k_rope/k_nope/v from latentT; rope+concat+rmsnorm k; kT transpose
  phase 3: per (head, 512-col sq chunk): scoresT = kT.T @ qT  [sk, sq] in
           PSUM; exp on ACT (scale 1/sqrt(DK), no max subtraction -- scores
           are bounded ~±6 so fp32 exp is safe); causal handled by skipping
           sk tiles > sq and one affine_select on the diagonal strip;
           oT += v.T @ probsT; rowsum via ones-vector matmul; normalize oT
           with broadcast reciprocal; out_partial = sum_h oT_h.T @ wo_h.
"""

import os
import sys
from contextlib import ExitStack

import numpy as np

for _p in ("/opt/trn_rl_repo", os.path.expanduser("~/.axon_site/_ro/trn_rl_repo")):
    if os.path.isdir(_p) and _p not in sys.path:
        sys.path.append(_p)

import concourse.bass as bass
import concourse.bacc as bacc
import concourse.mybir as mybir
import concourse.tile as tile
from concourse.masks import make_identity

F32 = mybir.dt.float32
F32R = mybir.dt.float32r
AX = mybir.AxisListType
ALU = mybir.AluOpType
ACTF = mybir.ActivationFunctionType

# Problem constants (hardcoded for nn_MultiHeadLatentAttention_74904229642374)
B, S, D, H, DK, DL, DR = 2, 2048, 2048, 16, 128, 512, 64
EPS = 1e-6
NCORES = 8
GROUPS = 4            # head-group (tensor-parallel) dimension
HG = H // GROUPS      # heads per core
HD = HG * DK          # per-core head width of q/v/wo
PT = 128              # partition tile
SCH = 512             # sequence chunk width

USE_F32R = True
MMDT = F32R if USE_F32R else F32


def _r(ap):
    return ap


def build_nc(s=S, d=D, dl=DL, dbg=False, repeat=1):
    """Build the per-core Bass program. s/d/dl shrinkable for sim tests."""
    nsq = s // PT          # number of 128-row seq tiles
    nch = s // SCH         # number of 512-wide seq chunks
    nkd = d // PT          # D contraction chunks
    ndl = dl // PT         # DL contraction chunks
    spc = SCH // PT        # seq tiles per chunk (4)
    now = min(SCH, d)      # wo output chunk width
    nno = d // now         # output col chunks of wo matmul

    nc = bacc.Bacc("TRN2", target_bir_lowering=False, debug=False,
                   num_devices=NCORES)

    xt_d = nc.dram_tensor("xt", [d, s], MMDT, kind="ExternalInput")
    wq_d = nc.dram_tensor("wq", [d, HD], MMDT, kind="ExternalInput")
    wkv_d = nc.dram_tensor("wkv", [d, dl], MMDT, kind="ExternalInput")
    wkr_d = nc.dram_tensor("wkr", [dl, HG * DR], MMDT, kind="ExternalInput")
    wkn_d = nc.dram_tensor("wkn", [dl, HG * (DK - DR)], MMDT, kind="ExternalInput")
    wv_d = nc.dram_tensor("wv", [dl, HD], MMDT, kind="ExternalInput")
    wo_d = nc.dram_tensor("wo", [HD, d], MMDT, kind="ExternalInput")
    cos_d = nc.dram_tensor("cos", [s, DR // 2], F32, kind="ExternalInput")
    sin_d = nc.dram_tensor("sin", [s, DR // 2], F32, kind="ExternalInput")
    qnw_d = nc.dram_tensor("qnw", [DK, 1], F32, kind="ExternalInput")
    knw_d = nc.dram_tensor("knw", [DK, 1], F32, kind="ExternalInput")
    out_d = nc.dram_tensor("out", [s, d], F32, kind="ExternalOutput")
    lat_d = nc.dram_tensor("lat_spill", [dl, s], MMDT)  # internal DRAM
    if dbg:
        dbg_qt = nc.dram_tensor("dbg_qt", [PT, HG * s], MMDT, kind="ExternalOutput")
        dbg_kt = nc.dram_tensor("dbg_kt", [PT, HG * s], MMDT, kind="ExternalOutput")
        dbg_v = nc.dram_tensor("dbg_v", [PT, (s // PT) * HD], MMDT, kind="ExternalOutput")
        dbg_ot = nc.dram_tensor("dbg_ot", [PT, (s // SCH) * HG * SCH], MMDT, kind="ExternalOutput")
        dbg_rs = nc.dram_tensor("dbg_rs", [(s // SCH) * HG, SCH], F32, kind="ExternalOutput")

    scale = 1.0 / float(np.sqrt(np.float32(DK)))

    with tile.TileContext(nc) as tc:
      for _rep in range(repeat):
       with ExitStack() as ctx:
        const = ctx.enter_context(tc.tile_pool(name="const", bufs=1))
        qt_pool = ctx.enter_context(tc.tile_pool(name="qt", bufs=1))
        stat = ctx.enter_context(tc.tile_pool(name="stat", bufs=4))

        ident = const.tile([PT, PT], F32)
        make_identity(nc, ident[:])
        ones_f = const.tile([PT, 1], F32)
        nc.gpsimd.memset(ones_f[:], 1.0)
        ones = const.tile([PT, 1], MMDT)
        nc.vector.tensor_copy(ones[:], ones_f[:])
        qnw = const.tile([PT, 1], F32)
        nc.sync.dma_start(out=qnw[:], in_=qnw_d.ap())
        knw = const.tile([PT, 1], F32)
        nc.sync.dma_start(out=knw[:], in_=knw_d.ap())
        cos_sb = const.tile([PT, nsq, DR // 2], F32)
        sin_sb = const.tile([PT, nsq, DR // 2], F32)
        cos_r = cos_d.ap().rearrange("(t p) f -> t p f", p=PT)
        sin_r = sin_d.ap().rearrange("(t p) f -> t p f", p=PT)
        for t in range(nsq):
            nc.sync.dma_start(out=cos_sb[:, t, :], in_=cos_r[t])
            nc.sync.dma_start(out=sin_sb[:, t, :], in_=sin_r[t])

        qT = qt_pool.tile([PT, HG, s], MMDT)   # [dk, h, sq]

        def rmsnorm_rinv(psum_aps, width):
            """psum_aps: [128, HG, w] APs; returns rinv [128, HG] tile
            holding 1/sqrt(mean-over-DK of squares + eps). Uses ACT Square
            with accum_out (single PSUM read per op)."""
            parts = []
            for ap in psum_aps:
                pt_ = stat.tile([PT, HG], F32, tag="sspart")
                sq = stat.tile([PT, ap.shape[2]], F32, tag="sq")
                for h in range(HG):
                    nc.scalar.activation(sq[:], ap[:, h, :], ACTF.Square,
                                         accum_out=pt_[:, h:h + 1])
                parts.append(pt_)
            if len(parts) == 1:
                ss = parts[0]
            else:
                ss = stat.tile([PT, HG], F32, tag="ss")
                nc.vector.tensor_add(ss[:], parts[0][:], parts[1][:])
            nc.vector.tensor_scalar(ss[:], ss[:], 1.0 / DK, EPS,
                                    op0=ALU.mult, op1=ALU.add)
            nc.scalar.sqrt(ss[:], ss[:])
            rinv = stat.tile([PT, HG], F32, tag="rinv")
            nc.vector.reciprocal(rinv[:], ss[:])
            return rinv

        def rope_block(dst, x1, x2, st, tmp_pool):
            """dst [128, HG, DR] target; x1/x2 [128, HG, DR//2] first/second
            half source APs; st = seq tile index."""
            half = DR // 2
            c = cos_sb[:, st, :].unsqueeze(1).broadcast_to([PT, HG, half])
            sn = sin_sb[:, st, :].unsqueeze(1).broadcast_to([PT, HG, half])
            t1 = tmp_pool.tile([PT, HG, half], F32, tag="rt1")
            t2 = tmp_pool.tile([PT, HG, half], F32, tag="rt2")
            t3 = tmp_pool.tile([PT, HG, half], F32, tag="rt3")
            t4 = tmp_pool.tile([PT, HG, half], F32, tag="rt4")
            nc.vector.tensor_mul(t1[:], x1, c)
            nc.vector.tensor_mul(t2[:], x2, sn)
            nc.vector.tensor_mul(t3[:], x1, sn)
            nc.vector.tensor_mul(t4[:], x2, c)
            nc.vector.tensor_add(dst[:, :, 0:half], t1[:], t2[:])
            nc.vector.tensor_sub(dst[:, :, half:DR], t4[:], t3[:])

        # ---------------- phase 1: latentT (to DRAM) + qT ----------------
        with tc.tile_pool(name="wq", bufs=1) as wq_pool, \
             tc.tile_pool(name="wkv", bufs=1) as wkv_pool, \
             tc.tile_pool(name="xt", bufs=nkd + 2) as xt_pool, \
             tc.tile_pool(name="latst", bufs=3) as latst_pool, \
             tc.tile_pool(name="ps1", bufs=3, space="PSUM") as ps1, \
             tc.tile_pool(name="pst", bufs=2, space="PSUM") as pst, \
             tc.tile_pool(name="qn", bufs=2) as qn_pool, \
             tc.tile_pool(name="rtmp", bufs=2) as rtmp:

            wq_sb = wq_pool.tile([PT, nkd, HD], MMDT)
            wq_r = wq_d.ap().rearrange("(k p) n -> k p n", p=PT)
            wkv_sb = wkv_pool.tile([PT, nkd, dl], MMDT)
            wkv_r = wkv_d.ap().rearrange("(k p) n -> k p n", p=PT)
            for k in range(nkd):
                nc.sync.dma_start(out=wq_sb[:, k, :], in_=wq_r[k])
                nc.sync.dma_start(out=wkv_sb[:, k, :], in_=wkv_r[k])

            xt_r = xt_d.ap().rearrange("k (c ss) -> c k ss", ss=SCH) \
                .rearrange("c (k p) ss -> c k p ss", p=PT)
            for c in range(nch):
                xts = []
                for k in range(nkd):
                    xk = xt_pool.tile([PT, SCH], MMDT, tag="xt")
                    nc.sync.dma_start(out=xk[:], in_=xt_r[c, k])
                    xts.append(xk)
                for dt_ in range(ndl):
                    pl = ps1.tile([PT, SCH], F32, tag="pslat")
                    for k in range(nkd):
                        nc.tensor.matmul(
                            pl[:],
                            _r(wkv_sb[:, k, dt_ * PT:(dt_ + 1) * PT]),
                            _r(xts[k][:]),
                            start=(k == 0), stop=(k == nkd - 1))
                    lst = latst_pool.tile([PT, SCH], MMDT, tag="latst")
                    nc.scalar.copy(lst[:], pl[:])
                    nc.sync.dma_start(
                        out=lat_d.ap()[dt_ * PT:(dt_ + 1) * PT,
                                       c * SCH:(c + 1) * SCH],
                        in_=lst[:])
                for t in range(spc):
                    st = c * spc + t
                    pq = ps1.tile([PT, HD], F32, tag="psq")
                    for k in range(nkd):
                        nc.tensor.matmul(
                            pq[:],
                            _r(xts[k][:, t * PT:(t + 1) * PT]),
                            _r(wq_sb[:, k, :]),
                            start=(k == 0), stop=(k == nkd - 1))
                    pqv = pq[:].rearrange("p (h e) -> p h e", h=HG)
                    rinv = rmsnorm_rinv([pqv], DK)
                    qn = qn_pool.tile([PT, HG, DK], F32, tag="qn")
                    for h in range(HG):
                        nc.vector.tensor_scalar(
                            qn[:, h, :], pq[:, h * DK:(h + 1) * DK],
                            rinv[:, h:h + 1], None, op0=ALU.mult)
                    rope_block(qn[:, :, 0:DR], qn[:, :, 0:DR // 2],
                               qn[:, :, DR // 2:DR], st, rtmp)
                    for h in range(HG):
                        tp = pst.tile([PT, PT], F32, tag="tp")
                        nc.tensor.transpose(tp[:], qn[:, h, :], ident[:])
                        nc.vector.tensor_scalar(
                            qT[:, h, st * PT:(st + 1) * PT], tp[:],
                            qnw[:, 0:1], None, op0=ALU.mult)

        kt_pool = ctx.enter_context(tc.tile_pool(name="kt", bufs=1))
        v_pool = ctx.enter_context(tc.tile_pool(name="v", bufs=1))
        kT = kt_pool.tile([PT, HG, s], MMDT)         # [dk, h, sk]
        v_sb = v_pool.tile([PT, nsq, HG, DK], MMDT)  # [sk-in-tile, tile, h, dk]

        # ---------------- phase 2: k and v from latentT ----------------
        with tc.tile_pool(name="wkr", bufs=1) as wkr_pool, \
             tc.tile_pool(name="wkn", bufs=1) as wkn_pool, \
             tc.tile_pool(name="wv", bufs=1) as wv_pool, \
             tc.tile_pool(name="lat2", bufs=ndl + 2) as lat2_pool, \
             tc.tile_pool(name="ps2", bufs=2, space="PSUM") as ps2, \
             tc.tile_pool(name="pst2", bufs=2, space="PSUM") as pst2, \
             tc.tile_pool(name="kn", bufs=2) as kn_pool, \
             tc.tile_pool(name="rtmp2", bufs=2) as rtmp2:

            wkr_sb = wkr_pool.tile([PT, ndl, HG * DR], MMDT)
            wkr_r = wkr_d.ap().rearrange("(k p) n -> k p n", p=PT)
            wkn_sb = wkn_pool.tile([PT, ndl, HG * (DK - DR)], MMDT)
            wkn_r = wkn_d.ap().rearrange("(k p) n -> k p n", p=PT)
            wv_sb = wv_pool.tile([PT, ndl, HD], MMDT)
            wv_r = wv_d.ap().rearrange("(k p) n -> k p n", p=PT)
            for k in range(ndl):
                nc.sync.dma_start(out=wkr_sb[:, k, :], in_=wkr_r[k])
                nc.sync.dma_start(out=wkn_sb[:, k, :], in_=wkn_r[k])
                nc.sync.dma_start(out=wv_sb[:, k, :], in_=wv_r[k])

            lat_r = lat_d.ap().rearrange("k (c ss) -> c k ss", ss=SCH) \
                .rearrange("c (k p) ss -> c k p ss", p=PT)
            for c in range(nch):
                lts = []
                for k in range(ndl):
                    lk = lat2_pool.tile([PT, SCH], MMDT, tag="lat2")
                    nc.sync.dma_start(out=lk[:], in_=lat_r[c, k])
                    lts.append(lk)
                for t in range(spc):
                    st = c * spc + t
                    pkr = ps2.tile([PT, HG * DR], F32, tag="pskr")
                    pkn = ps2.tile([PT, HG * (DK - DR)], F32, tag="pskn")
                    pv = ps2.tile([PT, HD], F32, tag="psv")
                    for k in range(ndl):
                        lt = _r(lts[k][:, t * PT:(t + 1) * PT])
                        nc.tensor.matmul(pkr[:], lt, _r(wkr_sb[:, k, :]),
                                         start=(k == 0), stop=(k == ndl - 1))
                        nc.tensor.matmul(pkn[:], lt, _r(wkn_sb[:, k, :]),
                                         start=(k == 0), stop=(k == ndl - 1))
                        nc.tensor.matmul(pv[:], lt, _r(wv_sb[:, k, :]),
                                         start=(k == 0), stop=(k == ndl - 1))
                    pkrv = pkr[:].rearrange("p (h e) -> p h e", h=HG)
                    pknv = pkn[:].rearrange("p (h e) -> p h e", h=HG)
                    rinv = rmsnorm_rinv([pkrv, pknv], DK)
                    kn = kn_pool.tile([PT, HG, DK], F32, tag="kn")
                    rope_block(kn[:, :, 0:DR], pkrv[:, :, 0:DR // 2],
                               pkrv[:, :, DR // 2:DR], st, rtmp2)
                    nc.vector.tensor_copy(kn[:, :, DR:DK], pknv)
                    for h in range(HG):
                        nc.vector.tensor_scalar(
                            kn[:, h, :], kn[:, h, :],
                            rinv[:, h:h + 1], None, op0=ALU.mult)
                        tp = pst2.tile([PT, PT], F32, tag="tp2")
                        nc.tensor.transpose(tp[:], kn[:, h, :], ident[:])
                        nc.vector.tensor_scalar(
                            kT[:, h, st * PT:(st + 1) * PT], tp[:],
                            knw[:, 0:1], None, op0=ALU.mult)
                    nc.scalar.copy(
                        v_sb[:, st, :, :].rearrange("p h e -> p (h e)"), pv[:])

        if dbg:
            nc.sync.dma_start(out=dbg_qt.ap(),
                              in_=qT[:].rearrange("p h s -> p (h s)"))
            nc.sync.dma_start(out=dbg_kt.ap(),
                              in_=kT[:].rearrange("p h s -> p (h s)"))
            nc.sync.dma_start(out=dbg_v.ap(),
                              in_=v_sb[:].rearrange("p t h e -> p (t h e)"))

        # ---------------- phase 3: attention + wo ----------------
        with tc.tile_pool(name="wo", bufs=1) as wo_pool, \
             tc.tile_pool(name="probs", bufs=3) as probs_pool, \
             tc.tile_pool(name="ot", bufs=2) as ot_pool, \
             tc.tile_pool(name="bcast", bufs=2) as bc_pool, \
             tc.tile_pool(name="outst", bufs=3) as out_pool, \
             tc.tile_pool(name="pssc", bufs=2, space="PSUM") as pssc, \
             tc.tile_pool(name="psot", bufs=2, space="PSUM") as psot, \
             tc.tile_pool(name="psrs", bufs=2, space="PSUM") as psrs, \
             tc.tile_pool(name="pswo", bufs=2, space="PSUM") as pswo:

            wo_sb = wo_pool.tile([PT, HG, d], MMDT)
            wo_r = wo_d.ap().rearrange("(h p) n -> h p n", p=PT)
            for h in range(HG):
                nc.sync.dma_start(out=wo_sb[:, h, :], in_=wo_r[h])

            for cj in range(nch):
                ot_sb = ot_pool.tile([PT, HG, SCH], MMDT, tag="otsb")
                for h in range(HG):
                    po = psot.tile([PT, SCH], F32, tag="psot")
                    prs = psrs.tile([1, SCH], F32, tag="psrs")
                    nsk = spc * cj + spc
                    for i in range(nsk):
                        dg = i - spc * cj
                        c0 = 0 if dg < 0 else min(PT * dg, SCH - 256)
                        w = SCH - c0
                        psc = pssc.tile([PT, SCH], F32, tag="pssc")
                        nc.tensor.matmul(
                            psc[:, c0:SCH],
                            _r(kT[:, h, i * PT:(i + 1) * PT]),
                            _r(qT[:, h, cj * SCH + c0:(cj + 1) * SCH]),
                            start=True, stop=True)
                        pb = probs_pool.tile([PT, SCH], MMDT, tag="probs")
                        nc.scalar.activation(pb[:, c0:SCH], psc[:, c0:SCH],
                                             ACTF.Exp, scale=scale)
                        if dg >= 0:
                            nc.gpsimd.affine_select(
                                out=pb[:, c0:SCH], in_=pb[:, c0:SCH],
                                compare_op=ALU.is_ge, fill=0.0,
                                base=SCH * cj + c0 - PT * i,
                                pattern=[[1, w]], channel_multiplier=-1)
                        nc.tensor.matmul(
                            po[:, c0:SCH],
                            _r(v_sb[:, i, h, :]),
                            _r(pb[:, c0:SCH]),
                            start=(i == 0), stop=(i == nsk - 1))
                        nc.tensor.matmul(
                            prs[:, c0:SCH],
                            _r(ones[:]),
                            _r(pb[:, c0:SCH]),
                            start=(i == 0), stop=(i == nsk - 1))
                    # normalize oT by broadcast reciprocal of rowsums
                    rs_sb = bc_pool.tile([1, SCH], F32, tag="rssb")
                    nc.scalar.copy(rs_sb[:], prs[:])
                    bc = bc_pool.tile([PT, SCH], F32, tag="bcast")
                    nc.gpsimd.partition_broadcast(bc[:], rs_sb[:], channels=PT)
                    rec = bc_pool.tile([PT, SCH], F32, tag="rec")
                    nc.vector.reciprocal(rec[:], bc[:])
                    nc.vector.tensor_mul(ot_sb[:, h, :], po[:], rec[:])
                    if dbg:
                        nc.sync.dma_start(
                            out=dbg_rs.ap()[cj * HG + h:cj * HG + h + 1, :],
                            in_=rs_sb[:])
                if dbg:
                    nc.sync.dma_start(
                        out=dbg_ot.ap()[:, cj * HG * SCH:(cj + 1) * HG * SCH],
                        in_=ot_sb[:].rearrange("p h s -> p (h s)"))
                # wo for this chunk
                for t in range(spc):
                    st = cj * spc + t
                    for n in range(nno):
                        pw = pswo.tile([PT, now], F32, tag="pswo")
                        for h in range(HG):
                            nc.tensor.matmul(
                                pw[:],
                                _r(ot_sb[:, h, t * PT:(t + 1) * PT]),
                                _r(wo_sb[:, h, n * now:(n + 1) * now]),
                                start=(h == 0), stop=(h == HG - 1))
                        ob = out_pool.tile([PT, now], F32, tag="outst")
                        nc.scalar.copy(ob[:], pw[:])
                        nc.sync.dma_start(
                            out=out_d.ap()[st * PT:(st + 1) * PT,
                                           n * now:(n + 1) * now],
                            in_=ob[:])

    nc.compile()
    return nc


def rope_tables(s):
    quarter = DR // 4
    freq = (1.0 / 10000.0) ** np.linspace(0.0, 1.0, quarter, dtype=np.float32)
    freq = np.concatenate([freq, np.zeros((quarter,), np.float32)])
    theta = np.arange(s, dtype=np.float32)[:, None] * freq[None, :]
    return np.cos(theta).astype(np.float32), np.sin(theta).astype(np.float32)


def make_in_maps(x, wq, w_kv_down, w_k_rope, w_k_nope, wv, wo,
                 q_norm_w, k_norm_w):
    s = x.shape[1]
    cos, sin = rope_tables(s)
    ca = np.ascontiguousarray
    in_maps = []
    for c in range(NCORES):
        b, g = divmod(c, GROUPS)
        in_maps.append({
            "xt": ca(x[b].T).astype(np.float32),
            "wq": ca(wq[:, g * HD:(g + 1) * HD]),
            "wkv": ca(w_kv_down),
            "wkr": ca(w_k_rope[:, g * HG * DR:(g + 1) * HG * DR]),
            "wkn": ca(w_k_nope[:, g * HG * (DK - DR):(g + 1) * HG * (DK - DR)]),
            "wv": ca(wv[:, g * HD:(g + 1) * HD]),
            "wo": ca(wo[g * HD:(g + 1) * HD, :]),
            "cos": cos, "sin": sin,
            "qnw": ca(q_norm_w.reshape(DK, 1)),
            "knw": ca(k_norm_w.reshape(DK, 1)),
        })
    return in_maps


_NC_CACHE = {}


def run(inputs, trace=False, **kwargs):
    from concourse.bass_utils import run_bass_kernel_spmd
    if "nc" not in _NC_CACHE:
        _NC_CACHE["nc"] = build_nc()
    nc = _NC_CACHE["nc"]
    in_maps = make_in_maps(**inputs)
    res = run_bass_kernel_spmd(nc, in_maps, core_ids=list(range(NCORES)),
                               trace=trace, **kwargs)
    outs = [r["out"] for r in res.results]
    full = np.empty((B, S, D), np.float32)
    for b in range(B):
        full[b] = outs[b * GROUPS]
        for g in range(1, GROUPS):
            full[b] += outs[b * GROUPS + g]
    return full, res


def kernel(**inputs):
    out, _ = run(inputs)
    return out
